# revision 1
# baseline (speedup 1.0000x reference)
"""Trainium2 Bass kernel for nn_DA_conv (dynamic depthwise conv + CA attention).

Data-parallel over batch: 16 samples / 8 cores = 2 samples per core.
Partition layout: 128 partitions = (sample s in 0..1) x (channel c in 0..63).

Per-core pipeline over the 128x128 image (free dim = h*128+w, 16384 cols):
  - feat fp32 DMA'd in 2048-col blocks; DVE converts into a W-padded fp16
    image buffer pad1 [128, 128*130] (zero cols at w=-1, w=128 per row).
  - dynamic 3x3 depthwise conv: per-partition tap scalars kern_p[128, 9]
    computed on-device from deg via small PE GEMMs.
    Blocks 0..NPE-1: all 9 taps as PE diagonal-matmul accumulation into PSUM
      (lhsT = diag(kern_p[:, tap]) fp16, rhs = shifted pad1 view).
    Remaining blocks: taps on DVE/ACT (init tensor_scalar 4x, some
      scalar_tensor_tensor 1x, some ACT-mul + DVE tensor_tensor-add 2x).
  - leaky-relu(0.1) via ACT Prelu -> act16.
  - 1x1 conv: PE matmul with block-diag(W_conv.T) fp16, + residual
    att*feat via PE diag(att) matmul into the same PSUM accumulation.
  - epilogue: ACT Identity(psum + b_conv) fp32 -> SBUF -> DMA out.

kernel(**inputs) takes FULL numpy inputs, returns FULL [16,64,128,128] f32.
"""
import numpy as np
from contextlib import ExitStack

import concourse.bass as bass
import concourse.tile as tile
from concourse import bacc, mybir
from concourse.bass_utils import run_bass_kernel_spmd

F8 = mybir.dt.float8e4
F16 = mybir.dt.float16
F32 = mybir.dt.float32
AF = mybir.ActivationFunctionType
OP = mybir.AluOpType
DR = mybir.MatmulPerfMode.DoubleRow

N_CORES = 8
B, C, H, W = 16, 64, 128, 128
BC = B // N_CORES          # 2 samples per core
P = BC * C                 # 128 partitions
HW = H * W                 # 16384
DEG, RED = 512, 8
K = 3
WP = W + 2                 # fp16 padded row stride (130)
WP8 = 136                  # fp8 padded row stride (2*136 B is 16B-aligned for
                           # the DoubleRow Ko step)
KSCALE = 1024.0            # fp8 tap weights are kern*1024 (e4m3 range);
                           # undone exactly by lrelu scale=1/1024
BLK = 2048                 # block cols (16 image rows)
NBLK = HW // BLK           # 8
ROWS_PER_BLK = BLK // W    # 16
NPE = 6                    # blocks 0..NPE-1 use PE taps; rest DVE/ACT
# converts for the DVE blocks (6,7) and their row-halo neighbours (5) go
# first so the DVE tap chain can start early; PE blocks stream in after.
CONVERT_ORDER = [0, 5, 6, 7, 1, 2, 3, 4]
# compute emission: DVE blocks early (their tap chain is the long pole),
# PE blocks throughout (steady PE work keeps the HAM clock gate at 8/8)
COMPUTE_ORDER = [6, 0, 1, 7, 2, 3, 4, 5]
# tap order: first tap must be di=0 (full coverage); init DVE tap is (0,-1)
TAPS = [(0, -1), (0, 0), (0, 1), (-1, -1), (-1, 0), (-1, 1), (1, -1), (1, 0), (1, 1)]
# non-PE blocks: which taps go ACT-assisted (mul on ACT, add on DVE 2x).
AA_TAPS = {(0, 0), (-1, 0), (1, 0)}   # the 3 odd-offset taps (stt is 1x anyway;
                                      # keep DVE for even ones)

_CACHE = {}


def _tap_idx(di, dj):
    return TAPS.index((di, dj))


def _build():
    nc = bacc.Bacc("TRN2", target_bir_lowering=False, debug=False,
                   num_devices=N_CORES)
    feat = nc.declare_dram_parameter("feat", [BC, C, H, W], F32, isOutput=False)
    deg = nc.declare_dram_parameter("deg", [BC, DEG, 64], F32, isOutput=False)
    wcat = nc.declare_dram_parameter("wcat", [DEG, 128], F32, isOutput=False)
    wk1t = nc.declare_dram_parameter("wk1t", [C, RED], F32, isOutput=False)
    wk2t = nc.declare_dram_parameter("wk2t", [RED, C * K * K], F32, isOutput=False)
    wdu1t = nc.declare_dram_parameter("wdu1t", [C, RED], F32, isOutput=False)
    wdu2t = nc.declare_dram_parameter("wdu2t", [RED, C], F32, isOutput=False)
    w2blk = nc.declare_dram_parameter("w2blk", [P, P], F16, isOutput=False)
    bias_p = nc.declare_dram_parameter("bias_p", [P, 1], F32, isOutput=False)
    eye16 = nc.declare_dram_parameter("eye16", [P, P], F16, isOutput=False)
    out = nc.declare_dram_parameter("out", [BC, C, H, W], F32, isOutput=True)

    featv = feat.ap().rearrange("s c h w -> (s c) (h w)")
    outv = out.ap().rearrange("s c h w -> (s c) (h w)")
    kern2_dram = nc.dram_tensor("kern2_tmp", [BC, C * K * K], F32)
    att_dram = nc.dram_tensor("att_tmp", [C, BC], F32)

    with tile.TileContext(nc) as tc:
        with ExitStack() as ctx:
            # ---------------- persistent pools ----------------
            const = ctx.enter_context(tc.tile_pool(name="const", bufs=1))
            padp = ctx.enter_context(tc.tile_pool(name="padp", bufs=1))

            pad1 = padp.tile([P, H * WP], F16)       # W-padded fp16 image
            pad1v = pad1[:].rearrange("p (h w) -> p h w", w=WP)
            # fp8 copy with zero halo rows at image rows -1 and 128
            pad8 = padp.tile([P, (H + 2) * WP8], F8)
            pad8v = pad8[:].rearrange("p (h w) -> p h w", w=WP8)

            def pad8_ap(flat_off, dims):
                base = pad8[:]
                return bass.AP(base.tensor, base.offset + flat_off,
                               [list(base.ap[0])] + [list(d) for d in dims])

            w2blk_sb = const.tile([P, P], F16)
            nc.sync.dma_start(w2blk_sb[:], w2blk.ap())
            bias_sb = const.tile([P, 1], F32)
            nc.sync.dma_start(bias_sb[:], bias_p.ap())
            eye_sb = const.tile([P, P], F16)
            nc.sync.dma_start(eye_sb[:], eye16.ap())
            wcat_sb = const.tile([128, 4 * 128], F32)
            nc.sync.dma_start(
                wcat_sb[:].rearrange("p (t m) -> p t m", t=4),
                wcat.ap().rearrange("(t p) m -> p t m", p=128))
            wk1t_sb = const.tile([C, RED], F32)
            nc.sync.dma_start(wk1t_sb[:], wk1t.ap())
            wk2t_sb = const.tile([RED, C * K * K], F32)
            nc.sync.dma_start(wk2t_sb[:], wk2t.ap())
            wdu1t_sb = const.tile([C, RED], F32)
            nc.sync.dma_start(wdu1t_sb[:], wdu1t.ap())
            wdu2t_sb = const.tile([RED, C], F32)
            nc.sync.dma_start(wdu2t_sb[:], wdu2t.ap())

            kern_p = const.tile([P, K * K], F32)      # per-partition tap scalars
            kern1k = const.tile([P, K * K], F32)      # kern * KSCALE
            att_p = const.tile([P, 1], F32)
            eye8_sb = const.tile([P, P], F8)
            diag8 = const.tile([P, 3 * P], F8)        # singles: (0,-1),(0,0),(0,1)
            drlhs8 = const.tile([P, 3 * 2 * P], F8)   # pairs [(−1,dj),(+1,dj)]
            attd16 = const.tile([P, P], F16)

            # zero the pad columns (w=0 and w=129 of each padded row)
            nc.vector.memset(pad1v[:, :, 0:1], 0.0)
            nc.vector.memset(pad1v[:, :, WP - 1:WP], 0.0)
            nc.vector.memset(pad8v[:, :, 0:1], 0.0)
            nc.vector.memset(pad8v[:, :, W + 1:W + 2], 0.0)
            nc.vector.memset(pad8v[:, 0, :], 0.0)          # image row -1
            nc.vector.memset(pad8v[:, H + 1, :], 0.0)      # image row 128
            nc.vector.tensor_copy(eye8_sb[:], eye_sb[:])

            # ---------------- prologue: small GEMM chain ----------------
            # Warmup matmuls are interleaved between the latency-bound GEMM
            # steps: they keep the PE HAM activity monitor busy (so the clock
            # gate flips to 8/8 early) without delaying the chain.
            with ExitStack() as pctx:
                pro = pctx.enter_context(tc.tile_pool(name="pro", bufs=1))
                pps = pctx.enter_context(
                    tc.tile_pool(name="pps", bufs=1, space="PSUM"))

                wp = pps.tile([P, 512], F32)
                wl = pro.tile([P, P], F16)
                wr = pro.tile([P, 512], F16)
                nc.vector.memset(wl[:], 0.0)
                nc.vector.memset(wr[:], 0.0)

                def warm(n):
                    for _ in range(n):
                        nc.tensor.matmul(wp[:], wl[:], wr[:],
                                         start=True, stop=True)

                warm(8)
                dg = pro.tile([128, 2 * 256], F32)
                for s in range(BC):
                    nc.sync.dma_start(
                        dg[:, s * 256:(s + 1) * 256].rearrange(
                            "p (t f) -> p t f", t=4),
                        deg.ap()[s].rearrange("(t p) f -> p t f", p=128))
                # dvec[s, d=t*128+p] = mean_f deg -> dv[p, s*4+t]
                dv = pro.tile([128, 8], F32)
                nc.vector.tensor_reduce(
                    dv[:], dg[:].rearrange("p (s t f) -> p s t f", s=2, f=64),
                    axis=mybir.AxisListType.X, op=OP.add)
                nc.vector.tensor_scalar_mul(dv[:], dv[:], 1.0 / 64.0)
                dvv = dv[:].rearrange("p (s t) -> p t s", t=4)

                # f/fa = dvec @ [W_size|W_ac].T : psum [128, 2]
                warm(4)
                pf = pps.tile([128, 2], F32)
                for t in range(4):
                    nc.tensor.matmul(pf[:], wcat_sb[:, t * 128:(t + 1) * 128],
                                     dvv[:, t, :], start=(t == 0), stop=(t == 3))
                f_sb = pro.tile([C, 2], F32)
                nc.scalar.activation(f_sb[:], pf[0:C, :], AF.Copy)
                fa_sb = pro.tile([C, 2], F32)
                nc.scalar.activation(fa_sb[:], pf[C:2 * C, :], AF.Copy)

                # kern chain
                warm(4)
                ph1 = pps.tile([RED, 2], F32)
                nc.tensor.matmul(ph1[:], wk1t_sb[:], f_sb[:], start=True, stop=True)
                h1l = pro.tile([RED, 2], F32)
                nc.scalar.activation(h1l[:], ph1[:], AF.Prelu, alpha=0.1)
                warm(4)
                pk1 = pps.tile([2, 512], F32)
                nc.tensor.matmul(pk1[:], h1l[:], wk2t_sb[:, 0:512],
                                 start=True, stop=True)
                pk2 = pps.tile([2, 64], F32)
                nc.tensor.matmul(pk2[:], h1l[:], wk2t_sb[:, 512:576],
                                 start=True, stop=True)
                kern2 = pro.tile([2, 576], F32)
                nc.scalar.activation(kern2[:, 0:512], pk1[:], AF.Copy)
                nc.scalar.activation(kern2[:, 512:576], pk2[:], AF.Copy)
                nc.sync.dma_start(kern2_dram.ap(), kern2[:])
                for s in range(BC):
                    nc.sync.dma_start(
                        kern_p[s * C:(s + 1) * C, :],
                        kern2_dram.ap()[s].rearrange("(c t) -> c t", t=9))

                # attention chain
                warm(4)
                ph2 = pps.tile([RED, 2], F32)
                nc.tensor.matmul(ph2[:], wdu1t_sb[:], fa_sb[:], start=True, stop=True)
                h2l = pro.tile([RED, 2], F32)
                nc.scalar.activation(h2l[:], ph2[:], AF.Prelu, alpha=0.1)
                warm(4)
                pat = pps.tile([C, 2], F32)
                nc.tensor.matmul(pat[:], wdu2t_sb[:], h2l[:], start=True, stop=True)
                att_sb = pro.tile([C, 2], F32)
                nc.scalar.activation(att_sb[:], pat[:], AF.Sigmoid)
                nc.sync.dma_start(att_dram.ap(), att_sb[:])
                for s in range(BC):
                    nc.sync.dma_start(att_p[s * C:(s + 1) * C, :],
                                      att_dram.ap()[:, s:s + 1])

            # ---------------- main loop pools ----------------
            finp = ctx.enter_context(tc.tile_pool(name="finp", bufs=4))
            accp = ctx.enter_context(tc.tile_pool(name="accp", bufs=2))
            tmpp = ctx.enter_context(tc.tile_pool(name="tmpp", bufs=4))
            actp = ctx.enter_context(tc.tile_pool(name="actp", bufs=4))
            outp = ctx.enter_context(tc.tile_pool(name="outp", bufs=4))
            pdwp = ctx.enter_context(tc.tile_pool(name="pdw", bufs=2, space="PSUM"))
            pcvp = ctx.enter_context(tc.tile_pool(name="pcv", bufs=2, space="PSUM"))

            def pad_view(r0, r1, dj):
                """pad1 view of image rows [r0, r1), cols shifted by dj."""
                return pad1v[:, r0:r1, 1 + dj:1 + dj + W]

            def emit_diag_builds():
                # scaled fp8 diag matrices for the PE taps; emitted after the
                # early casts so those stream on DVE while kern_p is in flight
                nc.vector.tensor_scalar_mul(kern1k[:], kern_p[:], KSCALE)
                for j, dj in enumerate((-1, 0, 1)):
                    ti = _tap_idx(0, dj)
                    nc.vector.tensor_scalar(
                        diag8[:, j * P:(j + 1) * P], eye8_sb[:],
                        kern1k[:, ti:ti + 1], None, op0=OP.mult)
                    tlo, thi = _tap_idx(-1, dj), _tap_idx(1, dj)
                    nc.vector.tensor_scalar(
                        drlhs8[:, (2 * j) * P:(2 * j + 1) * P], eye8_sb[:],
                        kern1k[:, tlo:tlo + 1], None, op0=OP.mult)
                    nc.vector.tensor_scalar(
                        drlhs8[:, (2 * j + 1) * P:(2 * j + 2) * P], eye8_sb[:],
                        kern1k[:, thi:thi + 1], None, op0=OP.mult)
                nc.vector.tensor_scalar(
                    attd16[:], eye_sb[:], att_p[:], None, op0=OP.mult)

            # DMA-in + converts, emitted per block in interleaved order
            fins = {}
            done_diags = False
            for b in CONVERT_ORDER:
                if len(fins) == 4 and not done_diags:
                    emit_diag_builds()
                    done_diags = True
                fin = finp.tile([P, BLK], F32)
                nc.sync.dma_start(fin[:], featv[:, b * BLK:(b + 1) * BLK])
                fins[b] = fin
                r0 = b * ROWS_PER_BLK
                nc.vector.tensor_copy(
                    pad1v[:, r0:r0 + ROWS_PER_BLK, 1:1 + W],
                    fin[:].rearrange("p (r w) -> p r w", w=W))
                if b < NPE:
                    nc.vector.tensor_copy(
                        pad8v[:, r0 + 1:r0 + 1 + ROWS_PER_BLK, 1:1 + W],
                        fin[:].rearrange("p (r w) -> p r w", w=W))
                elif b == NPE:
                    # halo: PE block NPE-1's di=+1 taps read this block's row 0
                    nc.vector.tensor_copy(
                        pad8v[:, r0 + 1:r0 + 2, 1:1 + W],
                        fin[:].rearrange("p (r w) -> p r w", w=W)[:, 0:1, :])

            accs = {}       # b -> acc16 tile (DVE blocks, pre-lrelu)
            acts = {}       # b -> [act 1024-half aps] (PE blocks, post-lrelu)

            def emit_dve_taps(b):
                r0 = b * ROWS_PER_BLK
                r1 = r0 + ROWS_PER_BLK
                acc = accp.tile([P, BLK], F16, tag="acc")
                accv = acc[:].rearrange("p (r w) -> p r w", w=W)
                ti0 = _tap_idx(0, -1)
                # init: tap (0,-1), full coverage, tensor_scalar 4x
                nc.vector.tensor_scalar(
                    accv[:], pad_view(r0, r1, -1),
                    kern_p[:, ti0:ti0 + 1], None, op0=OP.mult)
                for ti, (di, dj) in enumerate(TAPS):
                    if (di, dj) == (0, -1):
                        continue
                    a0, a1 = r0 + di, r1 + di
                    s0, s1 = max(a0, 0), min(a1, H)
                    o0 = s0 - a0
                    dst = accv[:, o0:o0 + (s1 - s0), :]
                    src = pad_view(s0, s1, dj)
                    if (di, dj) in AA_TAPS:
                        tmp = tmpp.tile([P, BLK], F16, tag="tmp")
                        tv = tmp[:].rearrange("p (r w) -> p r w", w=W)[
                            :, 0:(s1 - s0), :]
                        nc.scalar.activation(
                            tv, src, AF.Copy, scale=kern_p[:, ti:ti + 1])
                        nc.vector.tensor_tensor(dst, dst, tv, op=OP.add)
                    else:
                        nc.vector.scalar_tensor_tensor(
                            dst, src, kern_p[:, ti:ti + 1], dst,
                            op0=OP.mult, op1=OP.add)
                accs[b] = acc

            def emit_pe_taps(b):
                r0 = b * ROWS_PER_BLK
                halves = []
                for half in range(2):
                    pdw = pdwp.tile([P, 1024], F32)
                    pdwv = pdw[:].rearrange("p (r w) -> p r w", w=W)
                    for q in range(2):
                        c0 = r0 + half * 8 + q * 4   # first image row of chunk
                        dst = pdwv[:, q * 4:q * 4 + 4, :]
                        for j, dj in enumerate((-1, 0, 1)):
                            nc.tensor.matmul(
                                dst, diag8[:, j * P:(j + 1) * P],
                                pad8_ap((c0 + 1) * WP8 + 1 + dj,
                                        [[WP8, 4], [1, W]]),
                                start=(j == 0), stop=False)
                        for j, dj in enumerate((-1, 0, 1)):
                            nc.tensor.matmul(
                                dst,
                                drlhs8[:, 2 * j * P:2 * (j + 1) * P]
                                .rearrange("p (a m) -> p a m", a=2),
                                pad8_ap(c0 * WP8 + 1 + dj,
                                        [[2 * WP8, 2], [WP8, 4], [1, W]]),
                                start=False, stop=(j == 2),
                                perf_mode=DR)
                    act16 = actp.tile([P, 1024], F16, tag="act")
                    nc.scalar.activation(act16[:], pdw[:], AF.Prelu,
                                         alpha=0.1, scale=1.0 / KSCALE)
                    halves.append(act16[:])
                acts[b] = halves

            def emit_conv(b):
                r0 = b * ROWS_PER_BLK
                if b in accs:
                    act16b = actp.tile([P, BLK], F16, tag="actb")
                    nc.scalar.activation(act16b[:], accs[b][:], AF.Prelu,
                                         alpha=0.1)
                    halves = [act16b[:, 0:1024], act16b[:, 1024:2048]]
                else:
                    halves = acts[b]
                ostage = outp.tile([P, BLK], F32)
                for half in range(2):
                    at = halves[half]
                    pcv = pcvp.tile([P, 1024], F32)
                    for q in range(2):
                        c0 = r0 + half * 8 + q * 4
                        nc.tensor.matmul(
                            pcv[:, q * 512:(q + 1) * 512], w2blk_sb[:],
                            at[:, q * 512:(q + 1) * 512],
                            start=True, stop=False)
                        nc.tensor.matmul(
                            pcv[:, q * 512:(q + 1) * 512], attd16[:],
                            pad_view(c0, c0 + 4, 0),
                            start=False, stop=True)
                    nc.scalar.activation(
                        ostage[:, half * 1024:(half + 1) * 1024], pcv[:],
                        AF.Identity, bias=bias_sb[:], scale=1.0)
                nc.sync.dma_start(outv[:, b * BLK:(b + 1) * BLK], ostage[:])

            # stage 2: DVE tap chains (DVE + a few ACT muls only)
            for b in range(NPE, NBLK):
                emit_dve_taps(b)
            # stage 3: PE tap blocks with conv/epilogue interleaved so the
            # PE always has ready work and epilogues stream out early
            PE_SEQ = list(range(NPE))
            CONV_SEQ = [0, 1, 2, 6, 3, 4, 5, 7][:2 + NBLK - NPE] + []
            CONV_SEQ = [0, 1, 2, 6, 3, 4, 5, 7]
            ci = 0
            for idx, b in enumerate(PE_SEQ):
                emit_pe_taps(b)
                if idx >= 1:
                    emit_conv(CONV_SEQ[ci])
                    ci += 1
            while ci < NBLK:
                emit_conv(CONV_SEQ[ci])
                ci += 1

    nc.compile()
    return nc


def _prep_host(inputs):
    W_size = inputs["W_size"]
    W_ac = inputs["W_ac"]
    W_k1 = inputs["W_k1"]
    W_k2 = inputs["W_k2"]
    W_conv = inputs["W_conv"]
    b_conv = inputs["b_conv"]
    W_du1 = inputs["W_du1"]
    W_du2 = inputs["W_du2"]

    wcat = np.ascontiguousarray(
        np.concatenate([W_size, W_ac], axis=0).T.astype(np.float32))  # [512,128]
    wk1t = np.ascontiguousarray(W_k1.T.astype(np.float32))            # [64,8]
    wk2t = np.ascontiguousarray(W_k2.T.astype(np.float32))            # [8,576]
    wdu1t = np.ascontiguousarray(W_du1.T.astype(np.float32))          # [64,8]
    wdu2t = np.ascontiguousarray(W_du2.T.astype(np.float32))          # [8,64]
    w2blk = np.zeros((P, P), np.float16)
    wct = W_conv.T.astype(np.float16)                                  # [c, o]
    w2blk[0:C, 0:C] = wct
    w2blk[C:2 * C, C:2 * C] = wct
    bias_p = np.tile(b_conv.astype(np.float32), BC).reshape(P, 1)
    eye16 = np.eye(P, dtype=np.float16)
    return dict(wcat=wcat, wk1t=wk1t, wk2t=wk2t, wdu1t=wdu1t, wdu2t=wdu2t,
                w2blk=w2blk, bias_p=np.ascontiguousarray(bias_p), eye16=eye16)


def kernel(**inputs):
    if "nc" not in _CACHE:
        _CACHE["nc"] = _build()
    nc = _CACHE["nc"]

    shared = _prep_host(inputs)
    feat = np.ascontiguousarray(inputs["feat"].astype(np.float32, copy=False))
    deg = np.ascontiguousarray(
        inputs["deg"].astype(np.float32, copy=False).reshape(B, DEG, 64))

    in_maps = []
    for i in range(N_CORES):
        m = dict(shared)
        m["feat"] = feat[i * BC:(i + 1) * BC]
        m["deg"] = deg[i * BC:(i + 1) * BC]
        in_maps.append(m)

    res = None
    for attempt in range(3):
        try:
            res = run_bass_kernel_spmd(nc, in_maps, core_ids=list(range(N_CORES)))
            break
        except Exception:
            # first execution of a freshly compiled NEFF occasionally fails
            # with a transient device error; a retry succeeds
            if attempt == 2:
                raise
            import time
            time.sleep(5)
    out = np.concatenate([res.results[i]["out"] for i in range(N_CORES)], axis=0)
    return out.astype(np.float32)

